# revision 27
# baseline (speedup 1.0000x reference)
"""Trainium2 Bass kernel for nn_Model_26439818674684 — optimized v3.

Changes vs v2 (721 us):
  - Phase A: per-step 1-bank PSUM gate tiles split into [i,f] and [g,o]
    groups so sigmoid(i,f) starts after only 8 recurrence matmuls (not 16),
    natural torch gate order (no perm), x-matmuls prefetch 2 steps ahead
    into their own tiles (kills the alternating ~1.5us WAR stall on the PE
    queue that delayed every other burst), f32 activation outputs on the
    critical chain (avoids the f32->bf16 convert penalty).
  - Topic LSTM: same split-tile treatment; PSUM gate seeds (l0 input gates,
    l1 bias) via vector copies instead of scalar COPY ops (scalar was 82%
    busy and the bottleneck); l0(t)/l1(t-1) software pipeline kept.
  - Phase C attn: softmax computed on the [30 days, 20 topics] layout after
    the DRAM transpose (was [1, 600] single-partition, ~770ns/op), exp via
    tanh identity e^x=(1+tanh(x/2))/(1-tanh(x/2)) to stay in the
    sigmoid/tanh activation table (saves 2x 1.3us ACT_TABLE_LOAD), bf16
    weighted-sum ops.
  - Day LSTM: rp PSUM split into [i,f,o] + [g] banks with g-matmuls first
    so tanh(g) waits on 2 matmuls instead of 10; seeds via vector copies;
    yd save on vector; day/tail weight DMAs issued at kernel start.
  - Day attn: tanh-exp softmax (no table load), w2 matmul reads yd[64:128]
    directly (no DRAM round trip).
"""
import sys
sys.path.insert(0, '/opt/trn_rl_repo')

import numpy as np
import ml_dtypes

import concourse.bass as bass
import concourse.tile as tile
from concourse import bacc, mybir
from concourse.bass_utils import run_bass_kernel_spmd

F32 = mybir.dt.float32
BF16 = mybir.dt.bfloat16
AF = mybir.ActivationFunctionType
ALU = mybir.AluOpType
BF = ml_dtypes.bfloat16

NC_ = 8
DAYS, TOPICS, T, E, H, DH = 30, 20, 128, 300, 256, 64
B = DAYS * TOPICS          # 600
BC = B // NC_              # 75 sequences per core
EP = 384                   # E+bias padded to 3 K-tiles
NCH = T // 2               # 64 chunks of 2 steps

_cache = {}


def build():
    nc = bacc.Bacc("TRN2", target_bir_lowering=False, debug=False,
                   enable_asserts=False, num_devices=NC_)

    # ---------------- DRAM I/O ----------------
    # x: [chunk, part, k-tile, step-in-chunk, seq]
    x_d = nc.dram_tensor("x", [NCH, 128, 3, 2, BC], BF16, kind="ExternalInput")
    wih0_d = nc.dram_tensor("wih0", [128, 3, 4 * H], BF16, kind="ExternalInput")
    whh0_d = nc.dram_tensor("whh0", [128, 2, 4 * H], BF16, kind="ExternalInput")
    ones_p_d = nc.dram_tensor("ones_p", [128, 1], BF16, kind="ExternalInput")
    ones_f_d = nc.dram_tensor("ones_f", [1, 128], BF16, kind="ExternalInput")
    ones_f32_d = nc.dram_tensor("ones_f32", [1, 64], F32, kind="ExternalInput")
    t_wih0_d = nc.dram_tensor("t_wih0", [H, 4 * H], BF16, kind="ExternalInput")
    t_whh0_d = nc.dram_tensor("t_whh0", [H, 4 * H], BF16, kind="ExternalInput")
    t_wih1_d = nc.dram_tensor("t_wih1", [H, 4 * H], BF16, kind="ExternalInput")
    t_whh1_d = nc.dram_tensor("t_whh1", [H, 4 * H], BF16, kind="ExternalInput")
    t_b0_d = nc.dram_tensor("t_b0", [128, 8], F32, kind="ExternalInput")
    t_b1c_d = nc.dram_tensor("t_b1c", [128, 8], F32, kind="ExternalInput")
    w1t_d = nc.dram_tensor("w1t", [H, H], BF16, kind="ExternalInput")
    w1b_d = nc.dram_tensor("w1b", [128, 2], F32, kind="ExternalInput")
    d_wih0_d = nc.dram_tensor("d_wih0", [H, 4, DH], BF16, kind="ExternalInput")
    d_whh0_d = nc.dram_tensor("d_whh0", [DH, 4, DH], BF16, kind="ExternalInput")
    d_w1m_d = nc.dram_tensor("d_w1m", [128, 4, DH], BF16, kind="ExternalInput")
    d_b0_d = nc.dram_tensor("d_b0", [DH, 4], F32, kind="ExternalInput")
    d_b1_d = nc.dram_tensor("d_b1", [DH, 4], F32, kind="ExternalInput")
    w2t_d = nc.dram_tensor("w2t", [DH, DH], BF16, kind="ExternalInput")
    w2b_d = nc.dram_tensor("w2b", [DH, 1], F32, kind="ExternalInput")
    l1t_d = nc.dram_tensor("l1t", [DH, 48], F32, kind="ExternalInput")
    l1b_d = nc.dram_tensor("l1b", [48, 1], F32, kind="ExternalInput")
    l2t_d = nc.dram_tensor("l2t", [48, 16], F32, kind="ExternalInput")
    l2b_d = nc.dram_tensor("l2b", [16, 1], F32, kind="ExternalInput")
    hw16_d = nc.dram_tensor("hw16", [16, 4], F32, kind="ExternalInput")
    hw4_d = nc.dram_tensor("hw4", [4, 4], F32, kind="ExternalInput")
    hb_d = nc.dram_tensor("hb", [4, 1], F32, kind="ExternalInput")
    prev_d = nc.dram_tensor("prev", [4, 4], F32, kind="ExternalInput")
    res_d = nc.dram_tensor("res", [4, 1], F32, kind="ExternalOutput")

    with tile.TileContext(nc) as tc:
        with tc.tile_pool(name="persist", bufs=1) as pp, \
             tc.tile_pool(name="act", bufs=4) as ap_, \
             tc.tile_pool(name="dram", bufs=1, space="DRAM") as dp:

            # ---- all persistent weight DMAs issued up front ----
            wih = pp.tile([128, 3, 4 * H], BF16, tag="wih", name="wih")
            nc.sync.dma_start(wih[:], wih0_d.ap())
            whh = pp.tile([128, 2, 4 * H], BF16, tag="whh", name="whh")
            nc.sync.dma_start(whh[:], whh0_d.ap())
            # ======== Phase A: text LSTM layer 0, 75 sequences ========
            # gate m-tiles in natural torch order: 0..3 = iL,iH,fL,fH
            # (the "if" group), 4..7 = gL,gH,oL,oH (the "go" group).
            h = pp.tile([128, 2, BC], BF16, tag="h_txt", name="h")
            c = pp.tile([128, 2, BC], F32, tag="c_txt", name="c")
            nc.any.memset(h[:], 0.0)
            nc.any.memset(c[:], 0.0)

            ctxA = nc.named_scope("phaseA_text")
            ctxA.__enter__()
            with tc.tile_pool(name="xin", bufs=6) as xip, \
                 tc.tile_pool(name="gifp", bufs=3, space="PSUM") as gifp, \
                 tc.tile_pool(name="gogp", bufs=3, space="PSUM") as gogp, \
                 tc.tile_pool(name="aact", bufs=4) as aap:

                xt_tiles = {}
                gif_t = {}
                gog_t = {}

                def xdma(ch):
                    xt = xip.tile([128, 3, 2, BC], BF16, tag="xt", name="xt")
                    nc.sync.dma_start(xt[:], x_d.ap()[ch])
                    xt_tiles[ch] = xt

                def xmm(t):
                    ch, s = divmod(t, 2)
                    xt = xt_tiles[ch]
                    gif = gifp.tile([128, 4, BC], F32,
                                    padded_shape=[128, 4, 128],
                                    tag="gif", name="gif")
                    gog = gogp.tile([128, 4, BC], F32,
                                    padded_shape=[128, 4, 128],
                                    tag="gog", name="gog")
                    gif_t[t] = gif
                    gog_t[t] = gog
                    for m in range(4):
                        for k in range(3):
                            nc.tensor.matmul(
                                gif[:, m, :], wih[:, k, 128 * m:128 * (m + 1)],
                                xt[:, k, s, :], start=(k == 0), stop=False,
                                skip_group_check=True)
                    for m in range(4, 8):
                        for k in range(3):
                            nc.tensor.matmul(
                                gog[:, m - 4, :], wih[:, k, 128 * m:128 * (m + 1)],
                                xt[:, k, s, :], start=(k == 0), stop=False,
                                skip_group_check=True)
                    if s == 1:
                        xt_tiles.pop(ch)

                def burst(t):
                    gif, gog = gif_t[t], gog_t[t]
                    for m in range(4):
                        for k in range(2):
                            nc.tensor.matmul(
                                gif[:, m, :], whh[:, k, 128 * m:128 * (m + 1)],
                                h[:, k, :], start=False, stop=(k == 1),
                                skip_group_check=True)
                    for m in range(4, 8):   # g tiles first, then o tiles
                        for k in range(2):
                            nc.tensor.matmul(
                                gog[:, m - 4, :], whh[:, k, 128 * m:128 * (m + 1)],
                                h[:, k, :], start=False, stop=(k == 1),
                                skip_group_check=True)

                def acts(t):
                    gif = gif_t.pop(t)
                    gog = gog_t.pop(t)
                    sif = aap.tile([128, 4, BC], BF16, tag="sif", name="sif")
                    nc.scalar.activation(sif[:], gif[:, :, 0:BC], AF.Sigmoid)
                    tg = aap.tile([128, 2, BC], BF16, tag="tg", name="tg")
                    nc.scalar.activation(tg[:], gog[:, 0:2, 0:BC], AF.Tanh)
                    so = aap.tile([128, 2, BC], BF16, tag="so", name="so")
                    nc.scalar.activation(so[:], gog[:, 2:4, 0:BC], AF.Sigmoid)
                    nc.vector.tensor_mul(c[:], c[:], sif[:, 2:4, :])
                    tmp = aap.tile([128, 2, BC], BF16, tag="tmp", name="tmp")
                    nc.vector.tensor_mul(tmp[:], sif[:, 0:2, :], tg[:])
                    nc.vector.tensor_add(c[:], c[:], tmp[:])
                    tct = aap.tile([128, 2, BC], BF16, tag="tct", name="tct")
                    nc.scalar.activation(tct[:], c[:], AF.Tanh)
                    nc.vector.tensor_mul(h[:], so[:], tct[:])

                for _ch in range(8):
                    xdma(_ch)
                # small persistent weight DMAs ride the queue here: the
                # first 8 x chunks issue immediately, these drain while
                # phase A computes, the rest of the x chunks self-throttle
                ones_f = pp.tile([1, 128], BF16, tag="ones_f", name="ones_f")
                nc.sync.dma_start(ones_f[:], ones_f_d.ap())
                ones_p = pp.tile([128, 1], BF16, tag="ones_p", name="ones_p")
                nc.sync.dma_start(ones_p[:], ones_p_d.ap())
                tw = {}
                for nm, d_ in (("t_wih0", t_wih0_d), ("t_whh0", t_whh0_d),
                               ("t_wih1", t_wih1_d), ("t_whh1", t_whh1_d)):
                    tw[nm] = pp.tile([128, 2, 4 * H], BF16, tag=nm, name=nm)
                    nc.sync.dma_start(tw[nm][:],
                                      d_.ap().rearrange("(j p) m -> p j m", p=128))
                tb0 = pp.tile([128, 8], F32, tag="tb0", name="tb0")
                nc.sync.dma_start(tb0[:], t_b0_d.ap())
                t_b1c = pp.tile([128, 8], F32, tag="t_b1c", name="t_b1c")
                nc.sync.dma_start(t_b1c[:], t_b1c_d.ap())
                w1t = pp.tile([128, 2, H], BF16, tag="w1t", name="w1t")
                nc.sync.dma_start(w1t[:], w1t_d.ap().rearrange("(j p) m -> p j m", p=128))
                w1b = pp.tile([128, 2], F32, tag="w1b", name="w1b")
                nc.sync.dma_start(w1b[:], w1b_d.ap())
                dwih0 = pp.tile([128, 2, 4, DH], BF16, tag="dwih0", name="dwih0")
                nc.sync.dma_start(dwih0[:],
                                  d_wih0_d.ap().rearrange("(j p) g h -> p j g h", p=128))
                dwhh0 = pp.tile([DH, 4, DH], BF16, tag="dwhh0", name="dwhh0")
                nc.sync.dma_start(dwhh0[:], d_whh0_d.ap())
                dw1m = pp.tile([128, 4, DH], BF16, tag="dw1m", name="dw1m")
                nc.sync.dma_start(dw1m[:], d_w1m_d.ap())
                db0 = pp.tile([DH, 4], F32, tag="db0", name="db0")
                nc.sync.dma_start(db0[:], d_b0_d.ap())
                db1 = pp.tile([DH, 4], F32, tag="db1", name="db1")
                nc.sync.dma_start(db1[:], d_b1_d.ap())
                w2t = pp.tile([DH, DH], BF16, tag="w2t", name="w2t")
                nc.sync.dma_start(w2t[:], w2t_d.ap())
                w2b = pp.tile([DH, 1], F32, tag="w2b", name="w2b")
                nc.sync.dma_start(w2b[:], w2b_d.ap())
                ones64 = pp.tile([1, DH], F32, tag="ones64", name="ones64")
                nc.sync.dma_start(ones64[:], ones_f32_d.ap())
                l1t = pp.tile([DH, 48], F32, tag="l1t", name="l1t")
                nc.sync.dma_start(l1t[:], l1t_d.ap())
                l1b = pp.tile([48, 1], F32, tag="l1b", name="l1b")
                nc.sync.dma_start(l1b[:], l1b_d.ap())
                l2t = pp.tile([48, 16], F32, tag="l2t", name="l2t")
                nc.sync.dma_start(l2t[:], l2t_d.ap())
                l2b = pp.tile([16, 1], F32, tag="l2b", name="l2b")
                nc.sync.dma_start(l2b[:], l2b_d.ap())
                hw16 = pp.tile([16, 4], F32, tag="hw16", name="hw16")
                nc.sync.dma_start(hw16[:], hw16_d.ap())
                hw4 = pp.tile([4, 4], F32, tag="hw4", name="hw4")
                nc.sync.dma_start(hw4[:], hw4_d.ap())
                hb = pp.tile([4, 1], F32, tag="hb", name="hb")
                nc.sync.dma_start(hb[:], hb_d.ap())
                prev = pp.tile([4, 4], F32, tag="prev", name="prev")
                nc.sync.dma_start(prev[:], prev_d.ap())
                for _ch in range(8, NCH):
                    xdma(_ch)
                xmm(0)
                xmm(1)
                for t in range(T):
                    burst(t)
                    if t + 2 < T:
                        xmm(t + 2)
                    acts(t)

            ctxA.__exit__(None, None, None)
            # ======== Phase B: AllGather + topic LSTM ========
            ctxB = nc.named_scope("phaseB_gather")
            ctxB.__enter__()
            hl = dp.tile([2, 128, BC], BF16, tag="hl", name="hl")
            nc.sync.dma_start(hl.rearrange("j p b -> p j b"), h[:])
            gat = dp.tile([NC_, 2, 128, BC], BF16, tag="gat", name="gat")
            nc.gpsimd.collective_compute(
                "AllGather", ALU.bypass,
                replica_groups=[list(range(NC_))],
                ins=[hl.opt()], outs=[gat.opt()])
            h_all = pp.tile([128, 2, B], BF16, tag="h_all", name="h_all")
            for j_ in range(2):
                nc.sync.dma_start(
                    h_all[:, j_, :].rearrange("p (r b) -> p r b", r=NC_),
                    gat[:, j_].rearrange("r p b -> p r b"))

            ctxB.__exit__(None, None, None)
            ctxT = nc.named_scope("phaseB_topic")
            ctxT.__enter__()
            b1bc = pp.tile([128, 8, DAYS], F32, tag="b1bc", name="b1bc")
            nc.vector.tensor_copy(b1bc[:],
                                  t_b1c.unsqueeze(2).broadcast_to([128, 8, DAYS]))

            y0 = pp.tile([128, 2, TOPICS, DAYS], BF16, tag="y0", name="y0")
            ytop = pp.tile([128, 2, B], BF16, tag="ytop", name="ytop")
            z30 = pp.tile([128, 2, DAYS], BF16, tag="z30", name="z30")
            ct0 = pp.tile([128, 2, DAYS], F32, tag="ct0", name="ct0")
            ct1 = pp.tile([128, 2, DAYS], F32, tag="ct1", name="ct1")
            for ap0 in (z30, ct0, ct1):
                nc.any.memset(ap0[:], 0.0)
            ytop_r = ytop.rearrange("p j (d tp) -> p j tp d", tp=TOPICS)

            # L0 input gates over all 600 (day-major) columns
            gt0 = pp.tile([128, 8, B], BF16, tag="gt0", name="gt0")
            with tc.tile_pool(name="tpc", bufs=4, space="PSUM") as tpc:
                for nn in range(2):
                    cs = slice(300 * nn, 300 * (nn + 1))
                    for m in range(8):
                        pt = tpc.tile([128, 300], F32, padded_shape=[128, 512],
                                      tag="tp", name="pt")
                        for j in range(2):
                            nc.tensor.matmul(pt[:], tw["t_wih0"][:, j, 128 * m:128 * (m + 1)],
                                             h_all[:, j, cs], start=(j == 0), stop=(j == 1))
                        if m % 2 == 0:
                            nc.scalar.activation(gt0[:, m, cs], pt[:], AF.Identity,
                                                 bias=tb0[:, m:m + 1])
                        else:
                            nc.vector.tensor_scalar_add(gt0[:, m, cs], pt[:],
                                                        tb0[:, m:m + 1])
            gt0_r = gt0.rearrange("p m (d tp) -> p m tp d", tp=TOPICS)

            with tc.tile_pool(name="tifp", bufs=4, space="PSUM") as tifp, \
                 tc.tile_pool(name="togp", bufs=4, space="PSUM") as togp:

                tl_tiles = {}

                def t_mm_l0(t):
                    gif = tifp.tile([128, 4, DAYS], F32,
                                    padded_shape=[128, 4, 128],
                                    tag="tgif", name="tgif")
                    gog = togp.tile([128, 4, DAYS], F32,
                                    padded_shape=[128, 4, 128],
                                    tag="tgog", name="tgog")
                    tl_tiles[("l0", t)] = (gif, gog)
                    nc.vector.tensor_copy(gif[:], gt0_r[:, 0:4, t, :])
                    nc.vector.tensor_copy(gog[:], gt0_r[:, 4:8, t, :])
                    rhs = ((lambda j: z30[:, j, :]) if t == 0
                           else (lambda j, _t=t: y0[:, j, _t - 1, :]))
                    for m in range(4):
                        for j in range(2):
                            nc.tensor.matmul(
                                gif[:, m, :], tw["t_whh0"][:, j, 128 * m:128 * (m + 1)],
                                rhs(j), start=False, stop=(j == 1),
                                skip_group_check=True)
                    for m in range(4, 8):
                        for j in range(2):
                            nc.tensor.matmul(
                                gog[:, m - 4, :], tw["t_whh0"][:, j, 128 * m:128 * (m + 1)],
                                rhs(j), start=False, stop=(j == 1),
                                skip_group_check=True)

                def t_mm_l1(t):
                    gif = tifp.tile([128, 4, DAYS], F32,
                                    padded_shape=[128, 4, 128],
                                    tag="tgif", name="tgif1")
                    gog = togp.tile([128, 4, DAYS], F32,
                                    padded_shape=[128, 4, 128],
                                    tag="tgog", name="tgog1")
                    tl_tiles[("l1", t)] = (gif, gog)
                    nc.vector.tensor_copy(gif[:], b1bc[:, 0:4, :])
                    nc.vector.tensor_copy(gog[:], b1bc[:, 4:8, :])
                    rhs1 = ((lambda j: z30[:, j, :]) if t == 0
                            else (lambda j, _t=t: ytop_r[:, j, _t - 1, :]))
                    for m in range(4):
                        for j in range(2):
                            nc.tensor.matmul(
                                gif[:, m, :], tw["t_wih1"][:, j, 128 * m:128 * (m + 1)],
                                y0[:, j, t, :], start=False, stop=False,
                                skip_group_check=True)
                        for j in range(2):
                            nc.tensor.matmul(
                                gif[:, m, :], tw["t_whh1"][:, j, 128 * m:128 * (m + 1)],
                                rhs1(j), start=False, stop=(j == 1),
                                skip_group_check=True)
                    for m in range(4, 8):
                        for j in range(2):
                            nc.tensor.matmul(
                                gog[:, m - 4, :], tw["t_wih1"][:, j, 128 * m:128 * (m + 1)],
                                y0[:, j, t, :], start=False, stop=False,
                                skip_group_check=True)
                        for j in range(2):
                            nc.tensor.matmul(
                                gog[:, m - 4, :], tw["t_whh1"][:, j, 128 * m:128 * (m + 1)],
                                rhs1(j), start=False, stop=(j == 1),
                                skip_group_check=True)

                def t_acts(key, ct, out_ap):
                    gif, gog = tl_tiles.pop(key)
                    sif = ap_.tile([128, 4, DAYS], BF16, tag="t_sif", name="sif")
                    nc.scalar.activation(sif[:], gif[:, :, 0:DAYS], AF.Sigmoid)
                    tg = ap_.tile([128, 2, DAYS], BF16, tag="t_tg", name="tg")
                    nc.scalar.activation(tg[:], gog[:, 0:2, 0:DAYS], AF.Tanh)
                    so = ap_.tile([128, 2, DAYS], BF16, tag="t_so", name="so")
                    nc.scalar.activation(so[:], gog[:, 2:4, 0:DAYS], AF.Sigmoid)
                    nc.vector.tensor_mul(ct[:], ct[:], sif[:, 2:4, :])
                    tmp = ap_.tile([128, 2, DAYS], BF16, tag="t_tmp", name="tmp")
                    nc.vector.tensor_mul(tmp[:], sif[:, 0:2, :], tg[:])
                    nc.vector.tensor_add(ct[:], ct[:], tmp[:])
                    tct = ap_.tile([128, 2, DAYS], BF16, tag="t_tct", name="tct")
                    nc.scalar.activation(tct[:], ct[:], AF.Tanh)
                    nc.vector.tensor_mul(out_ap, so[:], tct[:])

                t_mm_l0(0)
                t_acts(("l0", 0), ct0, y0[:, :, 0, :])
                for t in range(1, TOPICS):
                    t_mm_l0(t)
                    t_mm_l1(t - 1)
                    t_acts(("l0", t), ct0, y0[:, :, t, :])
                    t_acts(("l1", t - 1), ct1, ytop_r[:, :, t - 1, :])
                t_mm_l1(TOPICS - 1)
                t_acts(("l1", TOPICS - 1), ct1, ytop_r[:, :, TOPICS - 1, :])
            ctxT.__exit__(None, None, None)
            # ======== Phase C: topic attention ========
            ctxC = nc.named_scope("phaseC_attn")
            ctxC.__enter__()
            h_top = y0[:, :, TOPICS - 1, :]
            with tc.tile_pool(name="cps", bufs=2, space="PSUM") as cps, \
                 tc.tile_pool(name="scps", bufs=1, space="PSUM") as scps:
                z = pp.tile([128, 2, B], BF16, tag="z", name="z")
                for mi in range(2):
                    for nn in range(2):
                        cs = slice(300 * nn, 300 * (nn + 1))
                        pt = cps.tile([128, 300], F32, padded_shape=[128, 512],
                                      tag="zps", name="pt2")
                        for j in range(2):
                            nc.tensor.matmul(pt[:], w1t[:, j, 128 * mi:128 * (mi + 1)],
                                             ytop[:, j, cs], start=(j == 0), stop=(j == 1))
                        nc.vector.tensor_scalar_add(z[:, mi, cs], pt[:],
                                                    w1b[:, mi:mi + 1])
                prod = pp.tile([128, 2, B], BF16, tag="prod", name="prod")
                z_r = z.rearrange("p j (d tp) -> p j d tp", tp=TOPICS)
                prod_r = prod.rearrange("p j (d tp) -> p j d tp", tp=TOPICS)
                nc.vector.tensor_mul(
                    prod_r[:], z_r[:],
                    h_top.unsqueeze(3).broadcast_to([128, 2, DAYS, TOPICS]))
                sc_ps = scps.tile([1, 2, 512], F32, tag="sc", name="sc_ps")
                for nn in range(2):
                    for j in range(2):
                        nc.tensor.matmul(sc_ps[0:1, nn, 0:300], ones_p[:, 0:1],
                                         prod[:, j, 300 * nn:300 * (nn + 1)],
                                         start=(j == 0), stop=(j == 1))
                sc = pp.tile([1, B], F32, tag="sc_sb", name="sc")
                nc.scalar.activation(sc.rearrange("p (nn x) -> p nn x", nn=2),
                                     sc_ps[0:1, :, 0:300], AF.Copy)
                # transpose scores to [days, topics] via DRAM, then softmax
                # + keep-mask in the multi-partition layout
                d600 = dp.tile([B], F32, tag="d600", name="d600")
                nc.sync.dma_start(d600[:], sc[0:1, :])
                att_s = pp.tile([DAYS, TOPICS], F32, tag="att_s", name="att_s")
                nc.sync.dma_start(att_s[:], d600.rearrange("(d tp) -> d tp", d=DAYS))
                mx = pp.tile([DAYS, 1], F32, tag="mx", name="mx")
                nc.vector.tensor_reduce(mx[:], att_s[:], mybir.AxisListType.X, ALU.max)
                nmxh = pp.tile([DAYS, 1], F32, tag="nmxh", name="nmxh")
                nc.vector.tensor_scalar_mul(nmxh[:], mx[:], -0.5)
                # exp(x-mx) = (1+t)/(1-t), t = tanh((x-mx)/2): stays in the
                # sigmoid/tanh activation table (no ACT_TABLE_LOAD)
                th = pp.tile([DAYS, TOPICS], F32, tag="th", name="th")
                nc.scalar.activation(th[:], att_s[:], AF.Tanh,
                                     bias=nmxh[:, 0:1], scale=0.5)
                ea = pp.tile([DAYS, TOPICS], F32, tag="ea", name="ea")
                nc.vector.tensor_scalar_add(ea[:], th[:], 1.0)
                eb = pp.tile([DAYS, TOPICS], F32, tag="eb", name="eb")
                nc.vector.tensor_scalar(eb[:], th[:], -1.0, 1.0,
                                        op0=ALU.mult, op1=ALU.add)
                rb = pp.tile([DAYS, TOPICS], F32, tag="rb", name="rb")
                nc.vector.reciprocal(rb[:], eb[:])
                ex = pp.tile([DAYS, TOPICS], F32, tag="ex", name="ex")
                nc.vector.tensor_mul(ex[:], ea[:], rb[:])
                zs = pp.tile([DAYS, 1], F32, tag="zs", name="zs")
                nc.vector.tensor_reduce(zs[:], ex[:], mybir.AxisListType.X, ALU.add)
                rz = pp.tile([DAYS, 1], F32, tag="rz", name="rz")
                nc.vector.reciprocal(rz[:], zs[:])
                att_d = pp.tile([DAYS, TOPICS], F32, tag="att_d", name="att_d")
                nc.vector.tensor_scalar_mul(att_d[:], ex[:], rz[:, 0:1])
                # keep-mask: exclusive cumsum of sorted weights <= 0.8
                a_tp = att_d.unsqueeze(1).broadcast_to([DAYS, TOPICS, TOPICS])
                a_t = att_d.unsqueeze(2).broadcast_to([DAYS, TOPICS, TOPICS])
                gtm = pp.tile([DAYS, TOPICS, TOPICS], F32, tag="gtm", name="gtm")
                nc.vector.tensor_tensor(gtm[:], a_tp, a_t, ALU.is_gt)
                nc.vector.tensor_mul(gtm[:], gtm[:], a_tp)
                excl = pp.tile([DAYS, TOPICS], F32, tag="excl", name="excl")
                nc.vector.tensor_reduce(excl[:], gtm[:], mybir.AxisListType.X, ALU.add)
                keep = pp.tile([DAYS, TOPICS], F32, tag="keep", name="keep")
                nc.vector.tensor_scalar(keep[:], excl[:], 0.8, scalar2=None,
                                        op0=ALU.is_le)
                wgt = pp.tile([DAYS, TOPICS], BF16, tag="wgt", name="wgt")
                nc.vector.tensor_tensor(wgt[:], keep[:], att_d[:], ALU.mult)
                d600b = dp.tile([B], BF16, tag="d600b", name="d600b")
                nc.sync.dma_start(d600b[:], wgt[:])
                wfl = pp.tile([1, B], BF16, tag="wfl", name="wfl")
                nc.sync.dma_start(wfl[:], d600b.rearrange("(x) -> x").unsqueeze(0))
                wb = pp.tile([128, B], BF16, tag="wb", name="wb")
                for nn in range(2):
                    bb = cps.tile([128, 300], F32, padded_shape=[128, 512],
                                  tag="bc", name="bb")
                    nc.tensor.matmul(bb[:], ones_f[0:1, :],
                                     wfl[0:1, 300 * nn:300 * (nn + 1)],
                                     start=True, stop=True)
                    if nn == 0:
                        nc.scalar.activation(wb[:, 0:300], bb[:], AF.Copy)
                    else:
                        nc.vector.tensor_copy(wb[:, 300:600], bb[:])
                my = pp.tile([128, 2, B], BF16, tag="my", name="my")
                nc.vector.tensor_mul(my[:], ytop[:],
                                     wb.unsqueeze(1).broadcast_to([128, 2, B]))
                dh = pp.tile([128, 2, DAYS], F32, tag="dh", name="dh")
                nc.vector.tensor_reduce(
                    dh[:], my.rearrange("p j (d tp) -> p j d tp", tp=TOPICS),
                    mybir.AxisListType.X, ALU.add)

            ctxC.__exit__(None, None, None)
            # ======== Phase D: day LSTM (gate-in-free layout) + head ====
            ctxD = nc.named_scope("phaseD_day")
            ctxD.__enter__()
            with tc.tile_pool(name="dtail", bufs=1, space="PSUM") as dps, \
                 tc.tile_pool(name="rifop", bufs=3, space="PSUM") as rifop, \
                 tc.tile_pool(name="rgp", bufs=3, space="PSUM") as rgp:
                dh_bf = pp.tile([128, 2, DAYS], BF16, tag="dh_bf", name="dh_bf")
                nc.vector.tensor_copy(dh_bf[:], dh[:])
                # day l0 input gates for all 30 steps; gate cols [i, f, o, g]
                g0 = pp.tile([DH, 4, DAYS], F32, tag="gday0", name="g0")
                gps_ = dps.tile([DH, 4, DAYS], F32, padded_shape=[128, 4, 128],
                                tag="gd", name="gps_")
                for g in range(4):
                    for j in range(2):
                        nc.tensor.matmul(gps_[0:DH, g, :], dwih0[:, j, g, :],
                                         dh_bf[:, j, :], start=(j == 0), stop=(j == 1))
                for g in range(4):
                    nc.vector.tensor_scalar_add(g0[:, g, :], gps_[0:DH, g, :],
                                                db0[:, g:g + 1])
                st = pp.tile([128, 1], BF16, tag="st_day", name="st")
                nc.any.memset(st[:], 0.0)
                ydl = pp.tile([DH, DAYS], BF16, tag="ydl", name="ydl")
                cm = pp.tile([128, 1], F32, tag="cm_day", name="cm")
                nc.any.memset(cm[:], 0.0)

                def merged_step(t0, t1):
                    rifo = rifop.tile([128, 3], F32, padded_shape=[128, 512],
                                      tag="rifo", name="rifo")
                    rg = rgp.tile([128, 1], F32, padded_shape=[128, 512],
                                  tag="rg", name="rg")
                    p0 = 0 if t0 is not None else DH
                    p1 = 128 if t1 is not None else DH
                    # seeds via vector writes into PSUM
                    if t0 is not None:
                        nc.vector.tensor_copy(rifo[0:DH, :], g0[:, 0:3, t0])
                        nc.vector.tensor_copy(rg[0:DH, :], g0[:, 3:4, t0])
                    if t1 is not None:
                        nc.vector.tensor_copy(rifo[DH:128, :], db1[:, 0:3])
                        nc.vector.tensor_copy(rg[DH:128, :], db1[:, 3:4])
                    # recurrence matmuls: g first so tanh(g) unblocks early;
                    # grouped by layer (same-shape LDWEIGHTS pair/pipeline,
                    # alternating 64-row and 128-row loads serialize them)
                    if t0 is not None:
                        nc.tensor.matmul(rg[0:DH, 0:1], dwhh0[0:DH, 3, :],
                                         st[0:DH, 0:1], start=False, stop=True,
                                         skip_group_check=True)
                        for g in range(3):
                            nc.tensor.matmul(rifo[0:DH, g:g + 1], dwhh0[0:DH, g, :],
                                             st[0:DH, 0:1], start=False, stop=True,
                                             skip_group_check=True)
                    if t1 is not None:
                        nc.tensor.matmul(rg[DH:128, 0:1], dw1m[:, 3, :],
                                         st[:, 0:1], start=False, stop=True,
                                         skip_group_check=True)
                        for g in range(3):
                            nc.tensor.matmul(rifo[DH:128, g:g + 1], dw1m[:, g, :],
                                             st[:, 0:1], start=False, stop=True,
                                             skip_group_check=True)
                    # cell
                    tgd = ap_.tile([128, 1], F32, tag="tg_d", name="tgd")
                    nc.scalar.activation(tgd[p0:p1], rg[p0:p1, 0:1], AF.Tanh)
                    sio = ap_.tile([128, 3], F32, tag="sio_d", name="sio")
                    nc.scalar.activation(sio[p0:p1], rifo[p0:p1, :], AF.Sigmoid)
                    tmpd = ap_.tile([128, 1], F32, tag="tmp_d", name="tmpd")
                    nc.vector.tensor_mul(tmpd[p0:p1], sio[p0:p1, 0:1], tgd[p0:p1])
                    nc.vector.scalar_tensor_tensor(cm[p0:p1], cm[p0:p1],
                                                   sio[p0:p1, 1:2], tmpd[p0:p1],
                                                   op0=ALU.mult, op1=ALU.add)
                    tncd = ap_.tile([128, 1], F32, tag="tnc_d", name="tncd")
                    nc.scalar.activation(tncd[p0:p1], cm[p0:p1], AF.Tanh)
                    nc.vector.tensor_scalar_mul(st[p0:p1, 0:1], tncd[p0:p1],
                                                sio[p0:p1, 2:3])
                    if t1 is not None:
                        # partition-shift the l1 hidden to rows 0:64 via a
                        # tiny DMA that hides under the next step's acts
                        nc.sync.dma_start(ydl[0:DH, t1:t1 + 1], st[DH:128, 0:1])

                merged_step(0, None)
                for t in range(1, DAYS):
                    merged_step(t, t - 1)
                merged_step(None, DAYS - 1)
                hd = st[0:DH, 0:1]

                # day attention
                zp = dps.tile([DH, DAYS], F32, padded_shape=[128, 512],
                              tag="tail_ps", name="zp")
                nc.tensor.matmul(zp[0:DH, :], w2t[0:DH, :], ydl[0:DH, :],
                                 start=True, stop=True)
                z2 = pp.tile([DH, DAYS], F32, tag="z2", name="z2")
                nc.scalar.activation(z2[:], zp[0:DH, :], AF.Identity, bias=w2b[:, 0:1])
                p2 = pp.tile([DH, DAYS], F32, tag="p2", name="p2")
                nc.vector.tensor_mul(p2[:], z2[:], hd.broadcast_to([DH, DAYS]))
                onesp64 = pp.tile([DH, 1], F32, tag="onesp64", name="onesp64")
                nc.any.memset(onesp64[:], 1.0)
                s2p = dps.tile([1, DAYS], F32, padded_shape=[128, 512],
                               tag="tail_ps", name="s2p")
                nc.tensor.matmul(s2p[0:1, :], onesp64[0:DH, 0:1], p2[0:DH, :],
                                 start=True, stop=True)
                sc2 = pp.tile([1, DAYS], F32, tag="sc2", name="sc2")
                nc.scalar.activation(sc2[:], s2p[0:1, :], AF.Copy)
                mx2 = pp.tile([1, 1], F32, tag="mx2", name="mx2")
                nc.vector.tensor_reduce(mx2[:], sc2[:], mybir.AxisListType.X, ALU.max)
                nmx2h = pp.tile([1, 1], F32, tag="nmx2h", name="nmx2h")
                nc.vector.tensor_scalar_mul(nmx2h[:], mx2[:], -0.5)
                th2 = pp.tile([1, DAYS], F32, tag="th2", name="th2")
                nc.scalar.activation(th2[:], sc2[:], AF.Tanh,
                                     bias=nmx2h[0:1, 0:1], scale=0.5)
                e2a = pp.tile([1, DAYS], F32, tag="e2a", name="e2a")
                nc.vector.tensor_scalar_add(e2a[:], th2[:], 1.0)
                e2b = pp.tile([1, DAYS], F32, tag="e2b", name="e2b")
                nc.vector.tensor_scalar(e2b[:], th2[:], -1.0, 1.0,
                                        op0=ALU.mult, op1=ALU.add)
                r2b = pp.tile([1, DAYS], F32, tag="r2b", name="r2b")
                nc.vector.reciprocal(r2b[:], e2b[:])
                e2 = pp.tile([1, DAYS], F32, tag="e2", name="e2")
                nc.vector.tensor_mul(e2[:], e2a[:], r2b[:])
                z2s = pp.tile([1, 1], F32, tag="z2s", name="z2s")
                nc.vector.tensor_reduce(z2s[:], e2[:], mybir.AxisListType.X, ALU.add)
                rz2 = pp.tile([1, 1], F32, tag="rz2", name="rz2")
                nc.vector.reciprocal(rz2[:], z2s[:])
                at2 = pp.tile([1, DAYS], F32, tag="at2", name="at2")
                nc.vector.tensor_scalar_mul(at2[:], e2[:], rz2[0:1, 0:1])
                a2p = dps.tile([DH, DAYS], F32, padded_shape=[128, 512],
                               tag="tail_ps", name="a2p")
                nc.tensor.matmul(a2p[0:DH, :], ones64[0:1, :], at2[0:1, :],
                                 start=True, stop=True)
                my2 = pp.tile([DH, DAYS], F32, tag="my2", name="my2")
                nc.vector.tensor_mul(my2[:], ydl[0:DH, :], a2p[0:DH, :])
                ctx = pp.tile([DH, 1], F32, tag="ctx", name="ctx")
                nc.vector.tensor_reduce(ctx[:], my2[:], mybir.AxisListType.X, ALU.add)

                # head
                h1p = dps.tile([48, 1], F32, padded_shape=[128, 512],
                               tag="tail_ps", name="h1p")
                nc.tensor.matmul(h1p[0:48, :], l1t[0:DH, :], ctx[0:DH, 0:1],
                                 start=True, stop=True)
                h1 = pp.tile([48, 1], F32, tag="h1", name="h1")
                nc.scalar.activation(h1[:], h1p[0:48, :], AF.Identity, bias=l1b[:, 0:1])
                h2p = dps.tile([16, 1], F32, padded_shape=[128, 512],
                               tag="tail_ps", name="h2p")
                nc.tensor.matmul(h2p[0:16, :], l2t[0:48, :], h1[0:48, 0:1],
                                 start=True, stop=True)
                h2 = pp.tile([16, 1], F32, tag="h2", name="h2")
                nc.scalar.activation(h2[:], h2p[0:16, :], AF.Identity, bias=l2b[:, 0:1])
                op_ = dps.tile([4, 1], F32, padded_shape=[128, 512],
                               tag="tail_ps", name="op_")
                nc.tensor.matmul(op_[0:4, :], hw16[0:16, :], h2[0:16, 0:1],
                                 start=True, stop=True)
                pv = pp.tile([4, 4], F32, tag="pv", name="pv")
                nc.vector.tensor_mul(pv[:], prev[:], hw4[:])
                pvs = pp.tile([4, 1], F32, tag="pvs", name="pvs")
                nc.vector.tensor_reduce(pvs[:], pv[:], mybir.AxisListType.X, ALU.add)
                r1 = pp.tile([4, 1], F32, tag="r1", name="r1")
                nc.vector.tensor_add(r1[:], op_[0:4, :], pvs[:])
                res_sb = pp.tile([4, 1], F32, tag="res_sb", name="res_sb")
                nc.vector.tensor_add(res_sb[:], r1[:], hb[:])
                nc.sync.dma_start(res_d.ap(), res_sb[:])
            ctxD.__exit__(None, None, None)

    nc.compile()
    return nc


# day-LSTM gate perm: torch [i, f, g, o] -> column order [i, f, o, g]
PERM_G4 = [0, 1, 3, 2]


def _prep(inputs):
    """Host-side sharding + layout prep. Text/topic gates stay in natural
    torch order [i, f, g, o] (m-tiles iL,iH,fL,fH,gL,gH,oL,oH)."""
    X = np.asarray(inputs["X"], np.float32)
    xf = X.reshape(B, T, E)
    shared = {}
    # text layer-0 weights, bias folded at row 300
    wihT = np.zeros((EP, 4 * H), np.float32)
    wihT[:E] = np.asarray(inputs["txt_Wih0"], np.float32).T
    wihT[E] = np.asarray(inputs["txt_b0"], np.float32)
    shared["wih0"] = np.ascontiguousarray(
        wihT.reshape(3, 128, 4 * H).transpose(1, 0, 2)).astype(BF)
    whhT = np.asarray(inputs["txt_Whh0"], np.float32).T
    shared["whh0"] = np.ascontiguousarray(
        whhT.reshape(2, 128, 4 * H).transpose(1, 0, 2)).astype(BF)
    shared["ones_p"] = np.ones((128, 1), BF)
    shared["ones_f"] = np.ones((1, 128), BF)
    shared["ones_f32"] = np.ones((1, 64), np.float32)
    for nm, w in (("t_wih0", "top_Wih0"), ("t_whh0", "top_Whh0"),
                  ("t_wih1", "top_Wih1"), ("t_whh1", "top_Whh1")):
        shared[nm] = np.asarray(inputs[w], np.float32).T.astype(BF)
    shared["t_b0"] = np.ascontiguousarray(
        np.asarray(inputs["top_b0"], np.float32).reshape(8, 128).T)
    shared["t_b1c"] = np.ascontiguousarray(
        np.asarray(inputs["top_b1"], np.float32).reshape(8, 128).T)
    shared["w1t"] = np.asarray(inputs["w1_W"], np.float32).T.astype(BF)
    shared["w1b"] = np.ascontiguousarray(
        np.asarray(inputs["w1_b"], np.float32).reshape(2, 128).T)
    for nm, w, kk in (("d_wih0", "day_Wih0", H), ("d_whh0", "day_Whh0", DH)):
        wm = np.asarray(inputs[w], np.float32)
        shared[nm] = np.ascontiguousarray(
            wm.reshape(4, DH, kk)[PERM_G4].transpose(2, 0, 1)).astype(BF)
    wi1 = np.asarray(inputs["day_Wih1"], np.float32).reshape(4, DH, DH)[PERM_G4]
    wh1 = np.asarray(inputs["day_Whh1"], np.float32).reshape(4, DH, DH)[PERM_G4]
    shared["d_w1m"] = np.ascontiguousarray(
        np.concatenate([wi1.transpose(2, 0, 1), wh1.transpose(2, 0, 1)],
                       axis=0)).astype(BF)
    shared["d_b0"] = np.ascontiguousarray(
        np.asarray(inputs["day_b0"], np.float32).reshape(4, DH)[PERM_G4].T)
    shared["d_b1"] = np.ascontiguousarray(
        np.asarray(inputs["day_b1"], np.float32).reshape(4, DH)[PERM_G4].T)
    shared["w2t"] = np.ascontiguousarray(
        np.asarray(inputs["w2_W"], np.float32).T).astype(BF)
    shared["w2b"] = np.asarray(inputs["w2_b"], np.float32).reshape(DH, 1)
    shared["l1t"] = np.ascontiguousarray(np.asarray(inputs["lin1_W"], np.float32).T)
    shared["l1b"] = np.asarray(inputs["lin1_b"], np.float32).reshape(48, 1)
    shared["l2t"] = np.ascontiguousarray(np.asarray(inputs["lin2_W"], np.float32).T)
    shared["l2b"] = np.asarray(inputs["lin2_b"], np.float32).reshape(16, 1)
    hw = np.asarray(inputs["head_W"], np.float32)
    shared["hw16"] = np.ascontiguousarray(hw[:, :16].T)
    shared["hw4"] = np.ascontiguousarray(hw[:, 16:])
    shared["hb"] = np.asarray(inputs["head_b"], np.float32).reshape(4, 1)
    shared["prev"] = np.asarray(inputs["previous_labels"], np.float32)

    in_maps = []
    for r in range(NC_):
        xr = xf[BC * r:BC * (r + 1)]                    # [75, 128, 300]
        xe = np.zeros((EP, T, BC), np.float32)
        xe[:E] = xr.transpose(2, 1, 0)
        xe[E] = 1.0
        # [ch, p, k, s, b]: xe[k*128+p, 2ch+s, b]
        xp = np.ascontiguousarray(
            xe.reshape(3, 128, NCH, 2, BC)
              .transpose(2, 1, 0, 3, 4)).astype(BF)
        m = dict(shared)
        m["x"] = xp
        in_maps.append(m)
    return in_maps


def kernel(**inputs) -> np.ndarray:
    if "nc" not in _cache:
        _cache["nc"] = build()
    nc = _cache["nc"]
    in_maps = _prep(inputs)
    import os
    trace = bool(os.environ.get("KERNEL_TRACE"))
    res = run_bass_kernel_spmd(nc, in_maps, core_ids=list(range(NC_)),
                               trace=trace)
    _cache["last_results"] = res
    return np.asarray(res.results[0]["res"], np.float32)
